# revision 12
# baseline (speedup 1.0000x reference)
"""GCNConv (transform + symmetric-norm aggregate + sigmoid) on 8 Trainium2 NeuronCores.

out_i = sigmoid(dinv_i * sum_{j->i} dinv_j*(xW)_j + dinv_i^2*(xW)_i + b),
dinv = 1/sqrt(1 + in_degree).

Device algorithm (SPMD over 8 cores; per-core differences are pure data):
  pass 0: dinv from CSR rowptr diffs (sub / sqrt / reciprocal on device)
  pass A: g = dinv * (x @ W) for all nodes on every core, in bf16 (x fed
          host-transposed as bf16; matmuls bf16). g stored in HBM as a
          PAIR-ROW table: row (p*392 + tp) = [g[2tp, p] | g[2tp+1, p]],
          i.e. 128 bf16 = 256B per row (the dma_gather minimum payload),
          50176 rows split into 2 int16-addressable windows (by p//64).
  pass B: per 128-dst-node tile: dma_gather of pair rows for the tile's
          dst-bucketed edge list (edges grouped by (window, tile-parity) so
          each 128-slot chunk reads one uniform 64-col half of the pair);
          one-hot S built on-device in bf16 (DVE is_equal of local-dst ids
          vs an iota row); segment-sum via bf16 PE matmuls accumulated in
          f32 PSUM (self-loop = identity one-hot chunk over own rows kept
          in SBUF from pass A); sigmoid(dinv*psum + b); store f32.

Each core's inputs are rotated by its tile offset so the program is address-
uniform: core c sees global node-tile (t + c*nt_core) % nt_pad at position t,
and its own output tiles are always tiles [0, nt_core).

Host side only re-formats data: COO->CSR bucket sort, padding, int16 index
encoding, x transpose + per-core rotation, dtype casts. All arithmetic runs
on device.
"""

import sys

for _p in ("/opt/trn_rl_repo", "/root/.axon_site/_ro/trn_rl_repo"):
    if _p not in sys.path:
        sys.path.append(_p)

import numpy as np
import ml_dtypes

import concourse.bacc as bacc
import concourse.bass as bass
import concourse.mybir as mybir
import concourse.tile as tile
from concourse.bass import ts
from concourse.bass_utils import run_bass_kernel_spmd

BF16 = ml_dtypes.bfloat16

P = 128
N_CORES = 8
BATCH_A = 8  # node tiles per pass-A iteration
TB_B = 7  # dst tiles per pass-B gather batch
NW = 2  # int16 index windows of the pair-row table (split on src partition)
NH = 2  # src-tile parity halves (selects 64-col half of the gathered pair)
NG = NW * NH  # edge groups per dst tile

GATHER128 = True  # gather 128B single-node payloads at 256B pair-row stride

_prog_cache: dict = {}


def _dma_gather_raw(
    eng,
    out_ap,
    in_ap,
    idxs_ap,
    num_idxs,
    num_idxs_reg,
    elem_size,
    elem_step,
    single_packet,
    queue_num,
):
    """nc.gpsimd.dma_gather with payload < 256B allowed (stride must still be
    a multiple of 256B -- the SDMA descriptor stride field is in 256B units;
    the descriptor *length* field is byte-granular)."""
    import concourse.ap_utils as ap_utils
    from concourse.bass import MemorySpace, exact_div, round_up_to_multiple

    eng._assert_queue_num(queue_num)
    assert idxs_ap.dtype == mybir.dt.int16
    assert in_ap.space == MemorySpace.DRAM
    assert idxs_ap.space == MemorySpace.SBUF and out_ap.space == MemorySpace.SBUF
    assert in_ap.dtype == out_ap.dtype
    assert ap_utils.ap_is_contiguous(out_ap.ap[1:])
    assert ap_utils.ap_is_contiguous(idxs_ap.ap[1:])
    assert in_ap.ap[-1][1] == out_ap.ap[-1][1] == elem_size
    assert out_ap.ap[0][1] * out_ap.ap[1][1] == round_up_to_multiple(num_idxs, 128)
    assert in_ap.ap[0][0] == elem_step
    stride_bytes = elem_step * mybir.dt.size(in_ap.dtype)
    stride_bytes_256 = exact_div(stride_bytes, 256)
    assert stride_bytes_256 < 256

    _in_ap = eng.lower_ap_dma(in_ap, for_custom_bir_dma=True)
    _idxs_ap = eng.lower_ap(idxs_ap)
    _out_ap = eng.lower_ap(out_ap)
    return eng.add_instruction(
        mybir.InstDMAGatherAnt(
            name=eng.bass.get_next_instruction_name(),
            ins=[*_in_ap, _idxs_ap, eng.lower_val_access(eng.to_reg(num_idxs_reg))],
            outs=[_out_ap],
            transpose=False,
            num_idxs=num_idxs,
            elem_size=elem_size,
            stride_bytes_256=stride_bytes_256,
            gen_mode=0,
            single_packet=single_packet,
            queue_num=queue_num,
            sbuf_tokens_per_rank=0,
            sbuf_free_dim_per_rank=0,
            sbuf_free_dim_pad_per_rank=0,
            sbuf_byte_offset=0,
        )
    )


def _plan(n_nodes: int):
    nt_real = -(-n_nodes // P)
    nt_pad = nt_real
    while (
        nt_pad % N_CORES
        or (nt_pad // N_CORES) % TB_B
        or nt_pad % BATCH_A
        or (nt_pad // N_CORES) % 2
    ):
        nt_pad += 1
    return nt_real, nt_pad, nt_pad * P, nt_pad // N_CORES


def preprocess(x: np.ndarray, edge_index: np.ndarray, W: np.ndarray, b: np.ndarray):
    n_nodes, hid = x.shape
    out_dim = W.shape[1]
    nt_real, nt_pad, npad, nt_core = _plan(n_nodes)
    tpn = nt_pad // 2  # pair rows per partition-group

    src = np.ascontiguousarray(edge_index[0]).astype(np.int64)
    dst = np.ascontiguousarray(edge_index[1]).astype(np.int64)
    e = src.shape[0]

    counts = np.bincount(dst, minlength=npad)
    rowptr = np.zeros(npad + 1, dtype=np.int64)
    np.cumsum(counts, out=rowptr[1:])

    # bucket edges by (dst tile, src window, src tile parity), stable
    tile_of = dst // P
    w_of = (src % P) // 64
    h_of = (src // P) % 2
    grp = tile_of * NG + w_of * NH + h_of
    order = np.argsort(grp, kind="stable")
    src_s = src[order]
    dst_s = dst[order]
    grp_s = grp[order]

    grp_counts = np.bincount(grp_s, minlength=nt_pad * NG)
    jq = int(max(1, -(-int(grp_counts.max()) // P)))  # chunks per group
    jc = NG * jq + 1  # chunks per tile incl. own/self-loop chunk
    slot_cap = jq * P

    grp_start = np.zeros(nt_pad * NG, dtype=np.int64)
    np.cumsum(grp_counts[:-1], out=grp_start[1:])
    pos = np.arange(e, dtype=np.int64) - grp_start[grp_s]
    slot = grp_s * slot_cap + pos

    # per-edge gather info (tile-rotation applied per core later)
    p64_s = (src_s % P) % 64
    tp_s = (src_s // P) // 2
    nslot = nt_pad * NG * slot_cap
    loc_p64 = np.zeros(nslot, dtype=np.int64)
    loc_tp = np.zeros(nslot, dtype=np.int64)
    dl_flat = np.full(nslot, -1.0, dtype=np.float32)
    loc_p64[slot] = p64_s
    loc_tp[slot] = tp_s
    dl_flat[slot] = (dst_s % P).astype(np.float32)

    loc_p64 = loc_p64.reshape(nt_pad, NW, NH, slot_cap)
    loc_tp = loc_tp.reshape(nt_pad, NW, NH, slot_cap)

    # dl input [P, nt_pad, jc]: chunk cc=((w*NH+h)*jq+j) at col t*jc+cc; own last
    dl4 = dl_flat.reshape(nt_pad, NG * jq, P)  # [t, cc, p]
    dl_all = np.empty((P, nt_pad, jc), dtype=np.float32)
    dl_all[:, :, : NG * jq] = dl4.transpose(2, 0, 1)
    dl_all[:, :, NG * jq] = np.arange(P, dtype=np.float32)[:, None]

    rp = rowptr.astype(np.float32)
    rp0 = rp[:npad].reshape(nt_pad, P).T.copy()
    rp1 = rp[1 : npad + 1].reshape(nt_pad, P).T.copy()

    xT = np.zeros((hid, npad), dtype=np.float32)
    xT[:, :n_nodes] = np.asarray(x, dtype=np.float32).T
    b_bcast = np.broadcast_to(np.asarray(b, np.float32), (P, out_dim)).copy()

    n_call = TB_B * slot_cap  # idxs per dma_gather call (one (window, parity))
    cols_call = n_call // 16
    nb = nt_core // TB_B

    shared = dict(
        W=np.asarray(W, np.float32).astype(BF16), b_bcast=b_bcast
    )
    per_core = []
    for c in range(N_CORES):
        t0 = c * nt_core
        xr = np.roll(xT, -t0 * P, axis=1).astype(BF16)
        r0 = np.roll(rp0, -t0, axis=1)
        r1 = np.roll(rp1, -t0, axis=1)
        dlc = np.ascontiguousarray(
            dl_all[:, t0 : t0 + nt_core, :].reshape(P, nt_core * jc)
        ).astype(BF16)
        # int16 window-local indices with rotated pair-tile index
        tp_rot = (loc_tp[t0 : t0 + nt_core] - t0 // 2) % tpn
        loc = (loc_p64[t0 : t0 + nt_core] * tpn + tp_rot).astype(np.int16)
        # calls: batch bb covers tiles [bb*TB_B, ...); call (bb, w, h) — one
        # per edge group, issued on its own SWDGE queue — concat of
        # [k (tile), slot] -> [nb * NG, n_call]
        loc_b = loc.reshape(nb, TB_B, NW, NH, slot_cap).transpose(0, 2, 3, 1, 4)
        loc_b = loc_b.reshape(nb * NG, n_call)
        # wrap each call: idx i -> [i%16, i//16]; stack calls on cols; x8
        wrapped = loc_b.reshape(nb * NG, cols_call, 16).transpose(0, 2, 1)
        idx16 = np.tile(
            wrapped.transpose(1, 0, 2).reshape(16, nb * NG * cols_call), (8, 1)
        )
        per_core.append(
            dict(
                xT=xr,
                rp0=r0,
                rp1=r1,
                dl=dlc,
                idx16=np.ascontiguousarray(idx16),
            )
        )
    meta = dict(
        n_nodes=n_nodes,
        hid=hid,
        out_dim=out_dim,
        nt_pad=nt_pad,
        npad=npad,
        nt_core=nt_core,
        jq=jq,
        jc=jc,
        tpn=tpn,
    )
    return meta, shared, per_core


def build_program(meta, variant="full"):
    hid, out_dim = meta["hid"], meta["out_dim"]
    nt_pad, nt_core = meta["nt_pad"], meta["nt_core"]
    jq, jc, tpn = meta["jq"], meta["jc"], meta["tpn"]
    npad = meta["npad"]
    f32, i32, i16 = mybir.dt.float32, mybir.dt.int32, mybir.dt.int16
    bf16 = mybir.dt.bfloat16

    slot_cap = jq * P
    n_call = TB_B * slot_cap
    cols_call = n_call // 16
    nb = nt_core // TB_B
    wrows = (P // NW) * tpn  # pair rows per window
    cpb = NG * TB_B * jq  # gathered chunks per batch

    nc = bacc.Bacc(
        "TRN2",
        target_bir_lowering=False,
        debug=False,
        num_devices=N_CORES,
        num_swdge_queues=4,
    )

    xT_d = nc.dram_tensor("xT", [hid, npad], bf16, kind="ExternalInput").ap()
    W_d = nc.dram_tensor("W", [hid, out_dim], bf16, kind="ExternalInput").ap()
    b_d = nc.dram_tensor("b_bcast", [P, out_dim], f32, kind="ExternalInput").ap()
    rp0_d = nc.dram_tensor("rp0", [P, nt_pad], f32, kind="ExternalInput").ap()
    rp1_d = nc.dram_tensor("rp1", [P, nt_pad], f32, kind="ExternalInput").ap()
    dl_d = nc.dram_tensor("dl", [P, nt_core * jc], bf16, kind="ExternalInput").ap()
    idx_d = nc.dram_tensor(
        "idx16", [P, nb * NG * cols_call], i16, kind="ExternalInput"
    ).ap()
    # g pair-row table: row (p*tpn + tp) = [g[2tp, p] | g[2tp+1, p]] (256B);
    # window w = rows [w*wrows, (w+1)*wrows) -- int16-addressable
    g_d = nc.dram_tensor("g", [P * tpn, 2 * out_dim], bf16, kind="Internal").ap()
    out_d = nc.dram_tensor("out", [nt_core * P, out_dim], f32, kind="ExternalOutput").ap()

    gw = g_d.rearrange("(p c) f -> p (c f)", p=P)
    gq_d = [g_d[ts(q, wrows), :] for q in range(NW)]

    with tile.TileContext(nc) as tc:
        with (
            tc.tile_pool(name="const", bufs=1) as const_pool,
            tc.tile_pool(name="work", bufs=3) as work,
            tc.tile_pool(name="gath", bufs=2) as gath_pool,
            tc.tile_pool(name="smat", bufs=3) as smat_pool,
            tc.tile_pool(name="psum", bufs=4, space="PSUM") as psum_pool,
        ):
            # ---- pass 0: constants + dinv ----
            W_sb = const_pool.tile([hid, out_dim], bf16)
            nc.sync.dma_start(W_sb[:], W_d[:])
            b_sb = const_pool.tile([P, out_dim], f32)
            nc.sync.dma_start(b_sb[:], b_d[:])

            dinv = const_pool.tile([P, nt_pad], f32)
            r0 = work.tile([P, nt_pad], f32, tag="rp")
            r1 = work.tile([P, nt_pad], f32, tag="rp")
            nc.sync.dma_start(r0[:], rp0_d[:])
            nc.sync.dma_start(r1[:], rp1_d[:])
            deg = work.tile([P, nt_pad], f32, tag="rp")
            nc.vector.scalar_tensor_tensor(
                out=deg[:],
                in0=r1[:],
                scalar=1.0,
                in1=r0[:],
                op0=mybir.AluOpType.add,
                op1=mybir.AluOpType.subtract,
            )
            sq = work.tile([P, nt_pad], f32, tag="rp")
            nc.scalar.activation(sq[:], deg[:], mybir.ActivationFunctionType.Sqrt)
            nc.vector.reciprocal(dinv[:], sq[:])

            iota_i = const_pool.tile([P, P], i32)
            nc.gpsimd.iota(iota_i[:], pattern=[[1, P]], base=0, channel_multiplier=0)
            iota_f = const_pool.tile([P, P], bf16)
            nc.vector.tensor_copy(iota_f[:], iota_i[:])

            # ---- pass A: g = dinv * (x @ W) for all node tiles (bf16) ----
            # own rows (this core's tiles t < nt_core) are captured into SBUF
            # on the way through, saving an HBM re-read in pass B
            own_sb = const_pool.tile([P, nt_core * out_dim], bf16)
            own_v = own_sb[:].rearrange("p (t d) -> p t d", d=out_dim)
            if variant != "noop":
                for tb in range(nt_pad // BATCH_A):
                    xt = work.tile([hid, BATCH_A * P], bf16, tag="xT")
                    nc.sync.dma_start(xt[:], xT_d[:, ts(tb, BATCH_A * P)])
                    hp = psum_pool.tile([P, BATCH_A * out_dim], f32, tag="psA")
                    for k in range(BATCH_A):
                        nc.tensor.matmul(
                            out=hp[:, ts(k, out_dim)],
                            lhsT=xt[:, ts(k, P)],
                            rhs=W_sb[:],
                            start=True,
                            stop=True,
                        )
                    gt = work.tile([P, BATCH_A, out_dim], bf16, tag="gA")
                    for k in range(BATCH_A):
                        nc.vector.tensor_scalar_mul(
                            gt[:, k, :],
                            hp[:, ts(k, out_dim)],
                            dinv[:, tb * BATCH_A + k : tb * BATCH_A + k + 1],
                        )
                    # batch covers pair-rows [tb*BATCH_A//2, +BATCH_A//2) of
                    # every partition: contiguous (c f) cols in the gw view
                    nc.scalar.dma_start(
                        gw[:, ts(tb, BATCH_A * out_dim)],
                        gt[:].rearrange("p k d -> p (k d)"),
                    )
                    lo = tb * BATCH_A
                    if lo < nt_core:
                        m = min(BATCH_A, nt_core - lo)
                        nc.vector.tensor_copy(own_v[:, lo : lo + m, :], gt[:, :m, :])

            # ---- pass B ----
            # probe flags: which pieces of pass B to emit
            do_gather = variant in ("g", "nomm", "full")
            do_smat = variant in ("nomm", "full")
            do_edge_mm = variant == "full"
            if variant in ("a", "noop", "g"):  # stub out writes
                zt = const_pool.tile([P, out_dim], f32)
                nc.vector.memset(zt[:], 0.5)
                for t in range(nt_core):
                    nc.scalar.dma_start(out_d[ts(t, P), :], zt[:])
            nb_eff = nb if variant not in ("a", "noop") else 0
            for bb in range(nb_eff):
                idx_sb = work.tile([P, NG * cols_call], i16, tag="idx")
                nc.sync.dma_start(idx_sb[:], idx_d[:, ts(bb, NG * cols_call)])
                dlb = work.tile([P, TB_B * jc], bf16, tag="dl")
                nc.sync.dma_start(dlb[:], dl_d[:, ts(bb, TB_B * jc)])
                if do_gather:
                    gdim = out_dim if GATHER128 else 2 * out_dim
                    gath = gath_pool.tile([P, cpb, gdim], bf16, tag="gath")
                    for q in range(NG):  # one call per (window, parity) group,
                        w, h = divmod(q, NH)  # on its own SWDGE queue / DMA ring
                        if GATHER128:
                            # gather only the needed 64-col half of each pair
                            # row: 128B payload at 256B row stride
                            _dma_gather_raw(
                                nc.gpsimd,
                                out_ap=gath[:, ts(q, TB_B * jq), :],
                                in_ap=gq_d[w][:, ts(h, out_dim)],
                                idxs_ap=idx_sb[:, ts(q, cols_call)],
                                num_idxs=n_call,
                                num_idxs_reg=n_call,
                                elem_size=out_dim,
                                elem_step=2 * out_dim,
                                single_packet=n_call <= 1024,
                                queue_num=q,
                            )
                        else:
                            nc.gpsimd.dma_gather(
                                out_ap=gath[:, ts(q, TB_B * jq), :],
                                in_ap=gq_d[w][:],
                                idxs_ap=idx_sb[:, ts(q, cols_call)],
                                num_idxs=n_call,
                                num_idxs_reg=n_call,
                                elem_size=2 * out_dim,
                                single_packet=n_call <= 1024,
                                queue_num=q,
                            )
                if not do_smat:
                    continue
                for k in range(TB_B):
                    t = bb * TB_B + k
                    dlt = dlb[:, ts(k, jc)]
                    S = smat_pool.tile([P, jc * P], bf16, tag="smat")
                    nc.vector.tensor_tensor(
                        out=S[:].rearrange("p (j q) -> p j q", j=jc),
                        in0=dlt[:, :, None].to_broadcast([P, jc, P]),
                        in1=iota_f[:, None, :].to_broadcast([P, jc, P]),
                        op=mybir.AluOpType.is_equal,
                    )
                    op = psum_pool.tile([P, out_dim], f32, tag="psB")
                    for cc in range(jc):
                        if cc < NG * jq:
                            if not do_edge_mm:
                                continue
                            q, j = divmod(cc, jq)  # q = w*NH + h
                            ch = q * TB_B * jq + k * jq + j
                            if GATHER128:
                                rhs = gath[:, ch, :]
                            else:
                                rhs = gath[:, ch, ts(q % NH, out_dim)]
                        else:
                            rhs = own_sb[:, ts(t, out_dim)]
                        nc.tensor.matmul(
                            out=op[:],
                            lhsT=S[:, ts(cc, P)],
                            rhs=rhs,
                            start=(cc == 0) if do_edge_mm else True,
                            stop=(cc == jc - 1),
                        )
                    ot = work.tile([P, out_dim], f32, tag="outt")
                    nc.vector.scalar_tensor_tensor(
                        out=ot[:],
                        in0=op[:],
                        scalar=dinv[:, t : t + 1],
                        in1=b_sb[:],
                        op0=mybir.AluOpType.mult,
                        op1=mybir.AluOpType.add,
                    )
                    osig = work.tile([P, out_dim], f32, tag="osig")
                    nc.scalar.activation(
                        osig[:], ot[:], mybir.ActivationFunctionType.Sigmoid
                    )
                    nc.scalar.dma_start(out_d[ts(t, P), :], osig[:])

    nc.compile()
    return nc


def _get_program(meta, variant="full"):
    key = (variant,) + tuple(sorted((k, v) for k, v in meta.items()))
    if key not in _prog_cache:
        _prog_cache[key] = build_program(meta, variant)
    return _prog_cache[key]


def make_in_maps(meta, shared, per_core):
    return [dict(shared, **per_core[c]) for c in range(N_CORES)]


def kernel(x, edge_index, W, b) -> np.ndarray:
    x = np.asarray(x, np.float32)
    edge_index = np.asarray(edge_index)
    W = np.asarray(W, np.float32)
    b = np.asarray(b, np.float32)

    meta, shared, per_core = preprocess(x, edge_index, W, b)
    nc = _get_program(meta)
    in_maps = make_in_maps(meta, shared, per_core)
    res = run_bass_kernel_spmd(nc, in_maps, core_ids=list(range(N_CORES)))
    outs = [res.results[c]["out"] for c in range(N_CORES)]
    full = np.concatenate(outs, axis=0)
    return np.asarray(full, np.float32)[: meta["n_nodes"]]


# revision 17
# speedup vs baseline: 1.2783x; 1.2783x over previous
"""GCNConv (transform + symmetric-norm aggregate + sigmoid) on 8 Trainium2 NeuronCores.

out_i = sigmoid(dinv_i * sum_{j->i} dinv_j*(xW)_j + dinv_i^2*(xW)_i + b),
dinv = 1/sqrt(1 + in_degree).

Device algorithm (SPMD over 8 cores; per-core differences are pure data):
  pass 0: dinv from CSR rowptr diffs (sub / sqrt / reciprocal on device)
  pass A: g = dinv * (x @ W) for all nodes on every core, in bf16 (x fed
          host-transposed as bf16; matmuls bf16). g stored in HBM as a
          PAIR-ROW table: row (p*392 + tp) = [g[2tp, p] | g[2tp+1, p]],
          128 bf16 = 256B per row, 50176 rows split into 2 int16-addressable
          windows (by p//64). Pair rows keep the pass-A writes contiguous
          (1KB/partition per batch) while halving the table vs f32.
  pass B: per 128-dst-node tile: dma_gather of g[src] rows for the tile's
          dst-bucketed edge list. Edges are grouped by (window, tile-parity)
          into 4 groups per tile; each group's call gathers 128B single-node
          payloads at 256B pair-row stride (in_ap column-sliced by parity;
          needs the raw InstDMAGatherAnt builder below since the library
          asserts 256B payloads) and runs on its OWN SWDGE queue: the 4 Q7
          cpu-pairs + DMA rings work in parallel, ~4x the gather bandwidth
          of the default single queue -- this was the whole kernel's
          bottleneck (one ring moves ~23 GB/s). One-hot S built on-device in
          bf16 (DVE is_equal of local-dst ids vs an iota row); segment-sum
          via bf16 PE matmuls accumulated in f32 PSUM (self-loop = identity
          one-hot chunk over own rows kept in SBUF from pass A);
          sigmoid(dinv*psum + b); store f32.

Each core's inputs are rotated by its tile offset so the program is address-
uniform: core c sees global node-tile (t + c*nt_core) % nt_pad at position t,
and its own output tiles are always tiles [0, nt_core).

Host side only re-formats data: COO->CSR bucket sort, padding, int16 index
encoding, x transpose + per-core rotation, dtype casts. All arithmetic runs
on device.
"""

import sys

for _p in ("/opt/trn_rl_repo", "/root/.axon_site/_ro/trn_rl_repo"):
    if _p not in sys.path:
        sys.path.append(_p)

import numpy as np
import ml_dtypes

import concourse.bacc as bacc
import concourse.bass as bass
import concourse.mybir as mybir
import concourse.tile as tile
from concourse.bass import ts
from concourse.bass_utils import run_bass_kernel_spmd

BF16 = ml_dtypes.bfloat16

P = 128
N_CORES = 8
BATCH_A = 8  # node tiles per pass-A iteration
TB_B = 7  # dst tiles per pass-B gather batch
NW = 2  # int16 index windows of the pair-row table (split on src partition)
NH = 2  # src-tile parity halves (selects 64-col half of the gathered pair)
NG = NW * NH  # edge groups per dst tile

GATHER128 = True  # gather 128B single-node payloads at 256B pair-row stride

_prog_cache: dict = {}


def _dma_gather_raw(
    eng,
    out_ap,
    in_ap,
    idxs_ap,
    num_idxs,
    num_idxs_reg,
    elem_size,
    elem_step,
    single_packet,
    queue_num,
):
    """nc.gpsimd.dma_gather with payload < 256B allowed (stride must still be
    a multiple of 256B -- the SDMA descriptor stride field is in 256B units;
    the descriptor *length* field is byte-granular)."""
    import concourse.ap_utils as ap_utils
    from concourse.bass import MemorySpace, exact_div, round_up_to_multiple

    eng._assert_queue_num(queue_num)
    assert idxs_ap.dtype == mybir.dt.int16
    assert in_ap.space == MemorySpace.DRAM
    assert idxs_ap.space == MemorySpace.SBUF and out_ap.space == MemorySpace.SBUF
    assert in_ap.dtype == out_ap.dtype
    assert ap_utils.ap_is_contiguous(out_ap.ap[1:])
    assert ap_utils.ap_is_contiguous(idxs_ap.ap[1:])
    assert in_ap.ap[-1][1] == out_ap.ap[-1][1] == elem_size
    assert out_ap.ap[0][1] * out_ap.ap[1][1] == round_up_to_multiple(num_idxs, 128)
    assert in_ap.ap[0][0] == elem_step
    stride_bytes = elem_step * mybir.dt.size(in_ap.dtype)
    stride_bytes_256 = exact_div(stride_bytes, 256)
    assert stride_bytes_256 < 256

    _in_ap = eng.lower_ap_dma(in_ap, for_custom_bir_dma=True)
    _idxs_ap = eng.lower_ap(idxs_ap)
    _out_ap = eng.lower_ap(out_ap)
    return eng.add_instruction(
        mybir.InstDMAGatherAnt(
            name=eng.bass.get_next_instruction_name(),
            ins=[*_in_ap, _idxs_ap, eng.lower_val_access(eng.to_reg(num_idxs_reg))],
            outs=[_out_ap],
            transpose=False,
            num_idxs=num_idxs,
            elem_size=elem_size,
            stride_bytes_256=stride_bytes_256,
            gen_mode=0,
            single_packet=single_packet,
            queue_num=queue_num,
            sbuf_tokens_per_rank=0,
            sbuf_free_dim_per_rank=0,
            sbuf_free_dim_pad_per_rank=0,
            sbuf_byte_offset=0,
        )
    )


def _plan(n_nodes: int):
    nt_real = -(-n_nodes // P)
    nt_pad = nt_real
    while (
        nt_pad % N_CORES
        or (nt_pad // N_CORES) % TB_B
        or nt_pad % BATCH_A
        or (nt_pad // N_CORES) % 2
    ):
        nt_pad += 1
    return nt_real, nt_pad, nt_pad * P, nt_pad // N_CORES


def preprocess(x: np.ndarray, edge_index: np.ndarray, W: np.ndarray, b: np.ndarray):
    n_nodes, hid = x.shape
    out_dim = W.shape[1]
    nt_real, nt_pad, npad, nt_core = _plan(n_nodes)
    tpn = nt_pad // 2  # pair rows per partition-group

    src = np.ascontiguousarray(edge_index[0]).astype(np.int64)
    dst = np.ascontiguousarray(edge_index[1]).astype(np.int64)
    e = src.shape[0]

    counts = np.bincount(dst, minlength=npad)
    rowptr = np.zeros(npad + 1, dtype=np.int64)
    np.cumsum(counts, out=rowptr[1:])

    # bucket edges by (dst tile, src window, src tile parity), stable
    tile_of = dst // P
    w_of = (src % P) // 64
    h_of = (src // P) % 2
    grp = tile_of * NG + w_of * NH + h_of
    order = np.argsort(grp, kind="stable")
    src_s = src[order]
    dst_s = dst[order]
    grp_s = grp[order]

    grp_counts = np.bincount(grp_s, minlength=nt_pad * NG)
    jq = int(max(1, -(-int(grp_counts.max()) // P)))  # chunks per group
    jc = NG * jq + 1  # chunks per tile incl. own/self-loop chunk
    slot_cap = jq * P

    grp_start = np.zeros(nt_pad * NG, dtype=np.int64)
    np.cumsum(grp_counts[:-1], out=grp_start[1:])
    pos = np.arange(e, dtype=np.int64) - grp_start[grp_s]
    slot = grp_s * slot_cap + pos

    # per-edge gather info (tile-rotation applied per core later)
    p64_s = (src_s % P) % 64
    tp_s = (src_s // P) // 2
    nslot = nt_pad * NG * slot_cap
    loc_p64 = np.zeros(nslot, dtype=np.int64)
    loc_tp = np.zeros(nslot, dtype=np.int64)
    dl_flat = np.full(nslot, -1.0, dtype=np.float32)
    loc_p64[slot] = p64_s
    loc_tp[slot] = tp_s
    dl_flat[slot] = (dst_s % P).astype(np.float32)

    loc_p64 = loc_p64.reshape(nt_pad, NW, NH, slot_cap)
    loc_tp = loc_tp.reshape(nt_pad, NW, NH, slot_cap)

    # dl input [P, nt_pad, jc]: chunk cc=((w*NH+h)*jq+j) at col t*jc+cc; own last
    dl4 = dl_flat.reshape(nt_pad, NG * jq, P)  # [t, cc, p]
    dl_all = np.empty((P, nt_pad, jc), dtype=np.float32)
    dl_all[:, :, : NG * jq] = dl4.transpose(2, 0, 1)
    dl_all[:, :, NG * jq] = np.arange(P, dtype=np.float32)[:, None]

    rp = rowptr.astype(np.float32)
    rp0 = rp[:npad].reshape(nt_pad, P).T.copy()
    rp1 = rp[1 : npad + 1].reshape(nt_pad, P).T.copy()

    xT = np.zeros((hid, npad), dtype=np.float32)
    xT[:, :n_nodes] = np.asarray(x, dtype=np.float32).T
    b_bcast = np.broadcast_to(np.asarray(b, np.float32), (P, out_dim)).copy()

    n_call = TB_B * slot_cap  # idxs per dma_gather call (one (window, parity))
    cols_call = n_call // 16
    nb = nt_core // TB_B

    shared = dict(
        W=np.asarray(W, np.float32).astype(BF16), b_bcast=b_bcast
    )
    per_core = []
    for c in range(N_CORES):
        t0 = c * nt_core
        xr = np.roll(xT, -t0 * P, axis=1).astype(BF16)
        r0 = np.roll(rp0, -t0, axis=1)
        r1 = np.roll(rp1, -t0, axis=1)
        dlc = np.ascontiguousarray(
            dl_all[:, t0 : t0 + nt_core, :].reshape(P, nt_core * jc)
        ).astype(BF16)
        # int16 window-local indices with rotated pair-tile index
        tp_rot = (loc_tp[t0 : t0 + nt_core] - t0 // 2) % tpn
        loc = (loc_p64[t0 : t0 + nt_core] * tpn + tp_rot).astype(np.int16)
        # calls: batch bb covers tiles [bb*TB_B, ...); call (bb, w, h) — one
        # per edge group, issued on its own SWDGE queue — concat of
        # [k (tile), slot] -> [nb * NG, n_call]
        loc_b = loc.reshape(nb, TB_B, NW, NH, slot_cap).transpose(0, 2, 3, 1, 4)
        loc_b = loc_b.reshape(nb * NG, n_call)
        # wrap each call: idx i -> [i%16, i//16]; stack calls on cols; x8
        wrapped = loc_b.reshape(nb * NG, cols_call, 16).transpose(0, 2, 1)
        idx16 = np.tile(
            wrapped.transpose(1, 0, 2).reshape(16, nb * NG * cols_call), (8, 1)
        )
        per_core.append(
            dict(
                xT=xr,
                rp0=r0,
                rp1=r1,
                dl=dlc,
                idx16=np.ascontiguousarray(idx16),
            )
        )
    meta = dict(
        n_nodes=n_nodes,
        hid=hid,
        out_dim=out_dim,
        nt_pad=nt_pad,
        npad=npad,
        nt_core=nt_core,
        jq=jq,
        jc=jc,
        tpn=tpn,
    )
    return meta, shared, per_core


def build_program(meta, variant="full"):
    hid, out_dim = meta["hid"], meta["out_dim"]
    nt_pad, nt_core = meta["nt_pad"], meta["nt_core"]
    jq, jc, tpn = meta["jq"], meta["jc"], meta["tpn"]
    npad = meta["npad"]
    f32, i32, i16 = mybir.dt.float32, mybir.dt.int32, mybir.dt.int16
    bf16 = mybir.dt.bfloat16

    slot_cap = jq * P
    n_call = TB_B * slot_cap
    cols_call = n_call // 16
    nb = nt_core // TB_B
    wrows = (P // NW) * tpn  # pair rows per window
    cpb = NG * TB_B * jq  # gathered chunks per batch

    nc = bacc.Bacc(
        "TRN2",
        target_bir_lowering=False,
        debug=False,
        num_devices=N_CORES,
        num_swdge_queues=4,
    )

    xT_d = nc.dram_tensor("xT", [hid, npad], bf16, kind="ExternalInput").ap()
    W_d = nc.dram_tensor("W", [hid, out_dim], bf16, kind="ExternalInput").ap()
    b_d = nc.dram_tensor("b_bcast", [P, out_dim], f32, kind="ExternalInput").ap()
    rp0_d = nc.dram_tensor("rp0", [P, nt_pad], f32, kind="ExternalInput").ap()
    rp1_d = nc.dram_tensor("rp1", [P, nt_pad], f32, kind="ExternalInput").ap()
    dl_d = nc.dram_tensor("dl", [P, nt_core * jc], bf16, kind="ExternalInput").ap()
    idx_d = nc.dram_tensor(
        "idx16", [P, nb * NG * cols_call], i16, kind="ExternalInput"
    ).ap()
    # g pair-row table: row (p*tpn + tp) = [g[2tp, p] | g[2tp+1, p]] (256B);
    # window w = rows [w*wrows, (w+1)*wrows) -- int16-addressable
    g_d = nc.dram_tensor("g", [P * tpn, 2 * out_dim], bf16, kind="Internal").ap()
    out_dt = bf16 if variant.endswith("16") else f32
    out_rows = P if variant == "tiny" else nt_core * P
    out_d = nc.dram_tensor("out", [out_rows, out_dim], out_dt, kind="ExternalOutput").ap()

    gw = g_d.rearrange("(p c) f -> p (c f)", p=P)
    gq_d = [g_d[ts(q, wrows), :] for q in range(NW)]

    with tile.TileContext(nc) as tc:
        with (
            tc.tile_pool(name="const", bufs=1) as const_pool,
            tc.tile_pool(name="work", bufs=3) as work,
            tc.tile_pool(name="gath", bufs=2) as gath_pool,
            tc.tile_pool(name="smat", bufs=3) as smat_pool,
            tc.tile_pool(name="psum", bufs=4, space="PSUM") as psum_pool,
        ):
            # ---- pass 0: constants + dinv ----
            W_sb = const_pool.tile([hid, out_dim], bf16)
            nc.sync.dma_start(W_sb[:], W_d[:])
            b_sb = const_pool.tile([P, out_dim], f32)
            nc.sync.dma_start(b_sb[:], b_d[:])

            dinv = const_pool.tile([P, nt_pad], f32)
            r0 = work.tile([P, nt_pad], f32, tag="rp")
            r1 = work.tile([P, nt_pad], f32, tag="rp")
            nc.sync.dma_start(r0[:], rp0_d[:])
            nc.sync.dma_start(r1[:], rp1_d[:])
            deg = work.tile([P, nt_pad], f32, tag="rp")
            nc.vector.scalar_tensor_tensor(
                out=deg[:],
                in0=r1[:],
                scalar=1.0,
                in1=r0[:],
                op0=mybir.AluOpType.add,
                op1=mybir.AluOpType.subtract,
            )
            sq = work.tile([P, nt_pad], f32, tag="rp")
            nc.scalar.activation(sq[:], deg[:], mybir.ActivationFunctionType.Sqrt)
            nc.vector.reciprocal(dinv[:], sq[:])

            iota_i = const_pool.tile([P, P], i32)
            nc.gpsimd.iota(iota_i[:], pattern=[[1, P]], base=0, channel_multiplier=0)
            iota_f = const_pool.tile([P, P], bf16)
            nc.vector.tensor_copy(iota_f[:], iota_i[:])

            # ---- pass A: g = dinv * (x @ W) for all node tiles (bf16) ----
            # own rows (this core's tiles t < nt_core) are captured into SBUF
            # on the way through, saving an HBM re-read in pass B
            own_sb = const_pool.tile([P, nt_core * out_dim], bf16)
            own_v = own_sb[:].rearrange("p (t d) -> p t d", d=out_dim)
            if variant != "noop":
                for tb in range(nt_pad // BATCH_A):
                    xt = work.tile([hid, BATCH_A * P], bf16, tag="xT")
                    nc.sync.dma_start(xt[:], xT_d[:, ts(tb, BATCH_A * P)])
                    hp = psum_pool.tile([P, BATCH_A * out_dim], f32, tag="psA")
                    for k in range(BATCH_A):
                        nc.tensor.matmul(
                            out=hp[:, ts(k, out_dim)],
                            lhsT=xt[:, ts(k, P)],
                            rhs=W_sb[:],
                            start=True,
                            stop=True,
                        )
                    gt = work.tile([P, BATCH_A, out_dim], bf16, tag="gA")
                    for k in range(BATCH_A):
                        nc.vector.tensor_scalar_mul(
                            gt[:, k, :],
                            hp[:, ts(k, out_dim)],
                            dinv[:, tb * BATCH_A + k : tb * BATCH_A + k + 1],
                        )
                    # batch covers pair-rows [tb*BATCH_A//2, +BATCH_A//2) of
                    # every partition: contiguous (c f) cols in the gw view
                    nc.scalar.dma_start(
                        gw[:, ts(tb, BATCH_A * out_dim)],
                        gt[:].rearrange("p k d -> p (k d)"),
                    )
                    lo = tb * BATCH_A
                    if lo < nt_core:
                        m = min(BATCH_A, nt_core - lo)
                        nc.vector.tensor_copy(own_v[:, lo : lo + m, :], gt[:, :m, :])

            # ---- pass B ----
            # probe flags: which pieces of pass B to emit
            do_gather = variant in ("g", "g256", "nomm", "full", "full16")
            do_smat = variant in ("nomm", "full", "full16")
            do_edge_mm = variant in ("full", "full16")
            gather128 = GATHER128 and variant != "g256"
            if variant in ("a", "noop", "noop16", "tiny", "g", "g256"):  # stub outs
                zt = const_pool.tile([P, out_dim], out_dt)
                nc.vector.memset(zt[:], 0.5)
                for t in range(1 if variant == "tiny" else nt_core):
                    nc.scalar.dma_start(out_d[ts(t, P), :], zt[:])
            nb_eff = nb if variant not in ("a", "noop", "noop16", "tiny") else 0
            for bb in range(nb_eff):
                idx_sb = work.tile([P, NG * cols_call], i16, tag="idx")
                nc.sync.dma_start(idx_sb[:], idx_d[:, ts(bb, NG * cols_call)])
                dlb = work.tile([P, TB_B * jc], bf16, tag="dl")
                nc.sync.dma_start(dlb[:], dl_d[:, ts(bb, TB_B * jc)])
                if do_gather:
                    gdim = out_dim if gather128 else 2 * out_dim
                    gath = gath_pool.tile([P, cpb, gdim], bf16, tag="gath")
                    for q in range(NG):  # one call per (window, parity) group,
                        w, h = divmod(q, NH)  # on its own SWDGE queue / DMA ring
                        if gather128:
                            # gather only the needed 64-col half of each pair
                            # row: 128B payload at 256B row stride
                            _dma_gather_raw(
                                nc.gpsimd,
                                out_ap=gath[:, ts(q, TB_B * jq), :],
                                in_ap=gq_d[w][:, ts(h, out_dim)],
                                idxs_ap=idx_sb[:, ts(q, cols_call)],
                                num_idxs=n_call,
                                num_idxs_reg=n_call,
                                elem_size=out_dim,
                                elem_step=2 * out_dim,
                                single_packet=n_call <= 1024,
                                queue_num=q,
                            )
                        else:
                            nc.gpsimd.dma_gather(
                                out_ap=gath[:, ts(q, TB_B * jq), :],
                                in_ap=gq_d[w][:],
                                idxs_ap=idx_sb[:, ts(q, cols_call)],
                                num_idxs=n_call,
                                num_idxs_reg=n_call,
                                elem_size=2 * out_dim,
                                single_packet=n_call <= 1024,
                                queue_num=q,
                            )
                if not do_smat:
                    continue
                for k in range(TB_B):
                    t = bb * TB_B + k
                    dlt = dlb[:, ts(k, jc)]
                    S = smat_pool.tile([P, jc * P], bf16, tag="smat")
                    nc.vector.tensor_tensor(
                        out=S[:].rearrange("p (j q) -> p j q", j=jc),
                        in0=dlt[:, :, None].to_broadcast([P, jc, P]),
                        in1=iota_f[:, None, :].to_broadcast([P, jc, P]),
                        op=mybir.AluOpType.is_equal,
                    )
                    op = psum_pool.tile([P, out_dim], f32, tag="psB")
                    for cc in range(jc):
                        if cc < NG * jq:
                            if not do_edge_mm:
                                continue
                            q, j = divmod(cc, jq)  # q = w*NH + h
                            ch = q * TB_B * jq + k * jq + j
                            if gather128:
                                rhs = gath[:, ch, :]
                            else:
                                rhs = gath[:, ch, ts(q % NH, out_dim)]
                        else:
                            rhs = own_sb[:, ts(t, out_dim)]
                        nc.tensor.matmul(
                            out=op[:],
                            lhsT=S[:, ts(cc, P)],
                            rhs=rhs,
                            start=(cc == 0) if do_edge_mm else True,
                            stop=(cc == jc - 1),
                        )
                    ot = work.tile([P, out_dim], f32, tag="outt")
                    nc.vector.scalar_tensor_tensor(
                        out=ot[:],
                        in0=op[:],
                        scalar=dinv[:, t : t + 1],
                        in1=b_sb[:],
                        op0=mybir.AluOpType.mult,
                        op1=mybir.AluOpType.add,
                    )
                    osig = work.tile([P, out_dim], out_dt, tag="osig")
                    nc.scalar.activation(
                        osig[:], ot[:], mybir.ActivationFunctionType.Sigmoid
                    )
                    nc.scalar.dma_start(out_d[ts(t, P), :], osig[:])

    nc.compile()
    return nc


def _get_program(meta, variant="full"):
    key = (variant,) + tuple(sorted((k, v) for k, v in meta.items()))
    if key not in _prog_cache:
        _prog_cache[key] = build_program(meta, variant)
    return _prog_cache[key]


def make_in_maps(meta, shared, per_core):
    return [dict(shared, **per_core[c]) for c in range(N_CORES)]


def kernel(x, edge_index, W, b) -> np.ndarray:
    x = np.asarray(x, np.float32)
    edge_index = np.asarray(edge_index)
    W = np.asarray(W, np.float32)
    b = np.asarray(b, np.float32)

    meta, shared, per_core = preprocess(x, edge_index, W, b)
    nc = _get_program(meta)
    in_maps = make_in_maps(meta, shared, per_core)
    res = run_bass_kernel_spmd(nc, in_maps, core_ids=list(range(N_CORES)))
    outs = [res.results[c]["out"] for c in range(N_CORES)]
    full = np.concatenate(outs, axis=0)
    return np.asarray(full, np.float32)[: meta["n_nodes"]]
